# revision 40
# baseline (speedup 1.0000x reference)
"""GroupQueryAttention (softmax over the GROUP axis) on 8 trn2 NeuronCores.

Reference computation (B=2, S=2048, D=1024, G=8, h=128):
    q = hidden @ Wq + bq ; k = hidden @ Wk + bk ; v = hidden @ Wv + bv
    scores[b,n,m,g] = sum_h q[b,n,g,h] k[b,m,g,h] / sqrt(D)
    probs = softmax(scores, axis=g)            # couples groups per (n,m)
    ctx[b,n,g,h] = sum_m probs[b,n,m,g] v[b,m,g,h]

Sharding: 2 batches x 4 query-blocks of 512 = 8 cores. The softmax over
g is local per core. Each core recomputes its batch's full K,V to avoid
cross-core collectives (~60us ncfw latency floor on this fabric).

Precision: Q,K projections run fp8e4 DoubleRowSwInterleave (weights
pre-interleaved on host so the fast-weight-load path stays on) with
x*32 / W*1024 pre-scales; V projection, scores and ctx matmuls stay
bf16 (an fp8 V or fp8 probs error enters ctx linearly through
sum_m p*dv and blows the max-abs gate).

Schedule: all K/V production is interleaved per-STAGE with the softmax
pipeline: each supertile stage emits its 4 score matmuls, then fillers
(one V psum chain, a 2-group K part on sts 1-3, and the previous
supertile's ctx matmuls), so the PE never outruns-and-stalls-on the
exp chain (scalar) and the HAM clock gate stays warm; ~62 dummy
matmuls at boot bridge the PE-activity window across the DMA-bound
prologue. Pass-1 softmax for supertiles 2-6 is precomputed during
pass 0 into retained E tiles (SBUF recycled from the fp8 projection
operands once K production ends at st3). Supertile 7 defers its mt15
V chains and its pass-1 precompute into the pass transition, where
they cover the exp->tree->normalize latency that the pass-0 drain
pieces wait on; the two fresh pass-1 chains then interleave with the
precomputed supertiles' ready ctx matmuls as stage fillers.

Softmax runs on 2-m-tile supertiles (8 x 512 probs): exp + V/ctx PSUM
evacuations on Scalar, pair-sum tree half on GpSimd (SBUF-only engine,
otherwise idle), tree tail + normalize mul on Vector. Q/K projection
evacuations alternate Scalar/Vector (scalar_tensor_tensor does
scale+bias on Vector) -- all-Vector during supertile 0 where scalar is
the ramp bottleneck -- so no single engine's serial evac chain paces
the DRSW projection matmuls.

Output: ctxT (1024, 512) bf16 per core; host upcasts/transposes/concats.
"""

import os

os.environ.setdefault("JAX_COMPILATION_CACHE_DIR", "/tmp/jax_comp_cache")

import numpy as np
import ml_dtypes

import concourse.bass as bass
import concourse.mybir as mybir
import concourse.tile as tile
from concourse import bacc
from concourse.bass_utils import run_bass_kernel_spmd

BF16 = mybir.dt.bfloat16
F32 = mybir.dt.float32
FP8 = mybir.dt.float8e4
DRSW = mybir.MatmulPerfMode.DoubleRowSwInterleave

B, S, D, G = 2, 2048, 1024, 8
H = D // G          # 128, group head dim
NQ = S // 4         # 512 queries per core
MT = S // 128       # 16 key m-tiles
ST = MT // 2        # 8 supertiles (2 m-tiles each)
CN = 256            # n-chunk (queries per attention pass)
NP = NQ // CN       # 2 passes
NPRE = 6            # pass-1 supertiles precomputed during pass 0
SCALE = 1.0 / np.sqrt(np.float32(D))  # 1/32
XS = 32.0           # fp8 pre-scale on x
WS = 1024.0         # fp8 pre-scale on Wq/Wk
DESC = 1.0 / (XS * WS)  # 2^-15 descale for fp8 QK psums

_CACHE = {}


def _sw_interleave(w8):
    """Host layout for DoubleRowSwInterleave stationary operands.

    w8: [128, 8, 1024] fp8 (partition, k-subtile t, out-col o). Returns
    [128, 4, 8, 256]: per (k-subtile-pair cp, out-group g of 128 cols),
    columns stored reversed with the (A=even subtile, B=odd subtile)
    values interleaved per column: pos 2*(127-c) = A[c], 2*(127-c)+1 = B[c].
    """
    A = w8[:, 0::2, :].reshape(128, 4, 8, 128)   # [p, cp, g, c]
    Bm = w8[:, 1::2, :].reshape(128, 4, 8, 128)
    inter = np.stack([A[..., ::-1], Bm[..., ::-1]], axis=-1)  # [p,cp,g,128,2]
    return np.ascontiguousarray(inter.reshape(128, 4, 8, 256))


def _build():
    nc = bacc.Bacc()

    xt_d = nc.dram_tensor("xt", [4, 128, 8, 512], BF16, kind="ExternalInput")
    xt8_d = nc.dram_tensor("xt8", [4, 128, 8, 512], FP8, kind="ExternalInput")
    wq8_d = nc.dram_tensor("wq8i", [128, 4, G, 256], FP8, kind="ExternalInput")
    wk8_d = nc.dram_tensor("wk8i", [128, 4, G, 256], FP8, kind="ExternalInput")
    wv_d = nc.dram_tensor("wv", [128, 8, D], BF16, kind="ExternalInput")
    bqs_d = nc.dram_tensor("bqs", [128, G], F32, kind="ExternalInput")
    bks_d = nc.dram_tensor("bks", [128, G], F32, kind="ExternalInput")
    bvt_d = nc.dram_tensor("bvt", [1, D], BF16, kind="ExternalInput")
    out_d = nc.dram_tensor("ctxT", [NP, 2, 128, 4, CN], BF16,
                           kind="ExternalOutput")

    with tile.TileContext(nc) as tc:
        with (
            tc.tile_pool(name="big", bufs=1) as big,
            tc.tile_pool(name="small", bufs=1) as small,
            tc.tile_pool(name="ework", bufs=2) as ework,
            tc.tile_pool(name="epre1", bufs=2) as epre1,
            tc.tile_pool(name="zwork", bufs=2) as zwork,
            tc.tile_pool(name="sc", bufs=2, space="PSUM") as scp,
            tc.tile_pool(name="cx", bufs=1, space="PSUM") as cxp,
        ):
            proj8 = tc.alloc_tile_pool(name="proj8", bufs=1)
            xt_s = big.tile([128, 4, 8, 512], BF16)  # [p, mc, dt, mcol]
            xt8_s = proj8.tile([128, 4, 8, 512], FP8)
            wq8_s = proj8.tile([128, 4, G, 256], FP8)
            wk8_s = proj8.tile([128, 4, G, 256], FP8)
            wv_s = big.tile([128, 8, D], BF16)

            # ---- input DMA, 3 queues, ordered by first-use time.
            # sync: the fp8 projection operands (K/Q critical path);
            # scalar: K weights then the xt bf16 chunks V needs first;
            # gpsimd: biases + V weights, then the late xt bulk.
            nc.sync.dma_start(xt8_s[:, 0], xt8_d[0])
            nc.sync.dma_start(wq8_s[:, :, 0:4], wq8_d[:, :, 0:4])
            nc.sync.dma_start(wq8_s[:, :, 4:8], wq8_d[:, :, 4:8])
            nc.sync.dma_start(xt8_s[:, 1], xt8_d[1])
            nc.sync.dma_start(
                xt8_s[:, 2:4], xt8_d[2:4].rearrange("c p t m -> p c t m")
            )
            nc.scalar.dma_start(wk8_s[:, :, 0:2], wk8_d[:, :, 0:2])
            nc.scalar.dma_start(wk8_s[:, :, 2:4], wk8_d[:, :, 2:4])
            nc.scalar.dma_start(wk8_s[:, :, 4:8], wk8_d[:, :, 4:8])
            nc.scalar.dma_start(xt_s[:, 0], xt_d[0])
            bqs_s = small.tile([128, G], F32)
            nc.gpsimd.dma_start(bqs_s[:], bqs_d[:])
            bks_s = small.tile([128, G], F32)
            nc.gpsimd.dma_start(bks_s[:], bks_d[:])
            bvt_s = small.tile([1, D], BF16)
            nc.gpsimd.dma_start(bvt_s[:], bvt_d[:])
            nc.gpsimd.dma_start(wv_s[:, :, 0:512], wv_d[:, :, 0:512])
            nc.gpsimd.dma_start(wv_s[:, :, 512:1024], wv_d[:, :, 512:1024])
            nc.gpsimd.dma_start(xt_s[:, 1], xt_d[1])
            nc.gpsimd.dma_start(
                xt_s[:, 2:4], xt_d[2:4].rearrange("c p t m -> p c t m")
            )
            ones_s = small.tile([1, 128], BF16)
            nc.gpsimd.memset(ones_s[:], 1.0)

            # ---- HAM warmup: ~50 dummy matmuls on a memset tile while
            # the input DMA is in flight. The PE clock gate (HAM) needs
            # ~3.4us of sustained activity to release the 1.2->2.4 GHz
            # throttle; without this the first ~7us of real matmuls run
            # at half clock.
            warm_s = small.tile([128, 128], BF16)
            nc.vector.memset(warm_s[:], 0.0)
            warmp = scp.tile([128, 64], F32, tag="sc")
            for w in range(62):
                nc.tensor.matmul(
                    warmp[:], warm_s[:], warm_s[:, 0:64],
                    start=(w == 0), stop=(w == 61),
                )

            kt_s = big.tile([128, G, S], BF16)       # [h, g, m]
            v_s = big.tile([128, MT, D], BF16)       # [m, mt, g*128+h]
            qt_s = big.tile([128, G, NQ], BF16)      # [h, g, n]
            ctxt_s = big.tile([128, G, CN], BF16)    # [h, g, n] one pass

            ident = mybir.ActivationFunctionType.Identity
            expf = mybir.ActivationFunctionType.Exp
            mult = mybir.AluOpType.mult
            addop = mybir.AluOpType.add

            def bias_bcast(bt, g, n):
                # column g of a [128, G] bias tile, broadcast over n cols
                return bass.AP(
                    tensor=bt.tensor, offset=bt.offset + g,
                    ap=[bt.ap[0], [0, n]],
                )

            # ---- Q^T projection (queries are XT columns 0:NQ), fp8 DRSW -----
            # Projection PSUM evacuations alternate scalar/vector so
            # neither engine's serial evac chain paces the DRSW matmuls.
            # During supertile 0 the ctx-accumulator banks are still idle;
            # boot_psum hands out sub-slots of them for every other
            # projection chain, widening the psum rotation from 2 to ~6
            # so no chain waits on an evac in the DMA-limited ramp.
            boot = {"t": None, "i": 0}
            allvec = {"on": False}

            def proj_psum(n):
                if boot["t"] is not None:
                    i = boot["i"]
                    boot["i"] += 1
                    if i % 2 == 1:
                        return boot["t"][:, (i // 2) % 4, 0:n]
                return scp.tile([128, n], F32, tag="sc", name="projp")

            def q_part(gh):
                for g in range(gh * 4, gh * 4 + 4):
                    qp = proj_psum(NQ)
                    for cp in range(4):
                        nc.tensor.matmul(
                            qp[:],
                            wq8_s[:, cp, g, :],
                            xt8_s[:, 0, 2 * cp : 2 * cp + 2, :],
                            start=(cp == 0),
                            stop=(cp == 3),
                            perf_mode=DRSW,
                        )
                    if g % 2 == 0 and not allvec["on"]:
                        nc.scalar.activation(
                            qt_s[:, g, :], qp[:], ident,
                            bias=bqs_s[:, g : g + 1],
                            scale=float(SCALE * DESC),
                        )
                    else:
                        nc.vector.scalar_tensor_tensor(
                            qt_s[:, g, :], qp[:], float(SCALE * DESC),
                            bias_bcast(bqs_s, g, NQ), mult, addop,
                        )

            def k_half(mc, g0):
                # K^T columns mc*512..+512 for groups g0, g0+1
                for g in (g0, g0 + 1):
                    kp = proj_psum(512)
                    for cp in range(4):
                        nc.tensor.matmul(
                            kp[:],
                            wk8_s[:, cp, g, :],
                            xt8_s[:, mc, 2 * cp : 2 * cp + 2, :],
                            start=(cp == 0),
                            stop=(cp == 3),
                            perf_mode=DRSW,
                        )
                    if g % 2 == 0 and not allvec["on"]:
                        nc.scalar.activation(
                            kt_s[:, g, mc * 512 : (mc + 1) * 512], kp[:],
                            ident, bias=bks_s[:, g : g + 1],
                            scale=float(DESC),
                        )
                    else:
                        nc.vector.scalar_tensor_tensor(
                            kt_s[:, g, mc * 512 : (mc + 1) * 512], kp[:],
                            float(DESC), bias_bcast(bks_s, g, 512),
                            mult, addop,
                        )

            def v_chain(mt, hc, evac_vec=True):  # noqa: D401
                # V rows for one (m-tile, 512-col half); +bv via a rank-1
                # ones matmul into the f32 PSUM (adding after the bf16
                # round would double the V quantization noise, which the
                # sum_m p*dv amplification turns into a gate failure)
                vp = scp.tile([128, 512], F32, tag="sc")
                for dt in range(8):
                    nc.tensor.matmul(
                        vp[:],
                        xt_s[:, mt // 4, dt,
                             (mt % 4) * 128 : (mt % 4) * 128 + 128],
                        wv_s[:, dt, hc * 512 : (hc + 1) * 512],
                        start=(dt == 0),
                        stop=False,
                    )
                nc.tensor.matmul(
                    vp[:],
                    ones_s[:],
                    bvt_s[:, hc * 512 : (hc + 1) * 512],
                    start=False,
                    stop=True,
                )
                # evac on vector: scalar's exp chain is the critical path
                # in the steady state and an extra 0.7us there surfaces
                # as a PE psum-rotation stall. st7 uses scalar instead so
                # the vector queue reaches st7's softmax tree sooner (the
                # pass-0 drain waits on it).
                if evac_vec:
                    nc.vector.tensor_copy(
                        v_s[:, mt, hc * 512 : (hc + 1) * 512], vp[:]
                    )
                else:
                    nc.scalar.activation(
                        v_s[:, mt, hc * 512 : (hc + 1) * 512], vp[:], ident
                    )

            def score_stage(np_, st, i, e_s):
                # 4 score matmuls + exp for stage i = (half, sub) of a
                # supertile against n-chunk np_.
                half, sub = divmod(i, 2)
                n0 = np_ * CN
                mt = 2 * st + sub
                sp = scp.tile([128, 4, CN], F32, tag="sc")
                for gl in range(4):
                    g = half * 4 + gl
                    nc.tensor.matmul(
                        sp[:, gl, :],
                        kt_s[:, g, mt * 128 : (mt + 1) * 128],
                        qt_s[:, g, n0 : n0 + CN],
                        start=True,
                        stop=True,
                    )
                nc.scalar.activation(
                    e_s[:, half * 4 : half * 4 + 4, sub * CN : (sub + 1) * CN],
                    sp[:], expf,
                )

            def pair_sum(i, e_s, t1):
                # pair-sums: half 0 on gpsimd (slack before t2 needs it),
                # half 1 on vector (fast, feeds t2 immediately)
                if i == 1:
                    nc.gpsimd.tensor_add(
                        t1[:, 0:2, :], e_s[:, 0:2, :], e_s[:, 2:4, :]
                    )
                elif i == 3:
                    nc.vector.tensor_add(
                        t1[:, 2:4, :], e_s[:, 4:6, :], e_s[:, 6:8, :]
                    )

            def tree_finish(e_s, t1):
                t2 = zwork.tile([128, 2, 2 * CN], BF16, tag="t2", bufs=1)
                nc.vector.tensor_add(t2[:], t1[:, 0:2, :], t1[:, 2:4, :])
                z32 = zwork.tile([128, 2 * CN], F32, tag="z32", bufs=1)
                nc.vector.tensor_add(z32[:], t2[:, 0, :], t2[:, 1, :])
                nc.vector.reciprocal_approx_fast(out=z32[:], in_=z32[:])
                wb = zwork.tile([128, 2 * CN], BF16, tag="wb", bufs=1)
                nc.vector.tensor_copy(wb[:], z32[:])
                # normalize per sub-tile so ctx matmuls on sub 0 can start
                # while sub 1 is still being scaled
                for sub in range(2):
                    wb_b = bass.AP(
                        tensor=wb.tensor, offset=wb.offset + sub * CN,
                        ap=[wb.ap[0], [0, G], [1, CN]],
                    )
                    nc.vector.tensor_mul(
                        e_s[:, :, sub * CN : (sub + 1) * CN],
                        e_s[:, :, sub * CN : (sub + 1) * CN],
                        wb_b,
                    )

            # ctx^T accumulation: out[h, n] += V_g^T @ P_g^T
            # Two groups share each 2KB PSUM bank. start=True resets the
            # whole bank's has_written bits, so only the first group of
            # each bank pair may issue it; the second group's first write
            # lands on cleared bits and overwrites, later writes accumulate.
            def ctx_piece(st_, e_, ctx_acc, half, sub, firsts, stop=False):
                mt = 2 * st_ + sub
                for g in range(half * 4, half * 4 + 4):
                    nc.tensor.matmul(
                        ctx_acc[:, g, :],
                        v_s[:, mt, g * 128 : (g + 1) * 128],
                        e_[:, g, sub * CN : (sub + 1) * CN],
                        start=(firsts and g % 2 == 0),
                        stop=stop,
                        skip_group_check=True,
                    )

            def evac_out(np_, gh, ctx_acc):
                nc.scalar.activation(
                    ctxt_s[:, gh * 4 : gh * 4 + 4, :],
                    ctx_acc[:, gh * 4 : gh * 4 + 4, :], ident,
                )
                nc.sync.dma_start(
                    out_d[np_, gh], ctxt_s[:, gh * 4 : gh * 4 + 4, :]
                )

            # =========== pass 0 ===========
            epre = {}

            # --- supertile 0 (special: projections are the fillers; V
            # chains go last so a late wv DMA can't block the PE queue)
            ctx_acc = cxp.tile([128, G, CN], F32, tag="cx")
            # st0: ALL Q/K evacs on vector -- scalar is the ramp
            # bottleneck (exps + V evacs + DMA issue), vector is idle
            allvec["on"] = True
            k_half(0, 0)
            k_half(0, 2)
            q_part(0)
            e0 = ework.tile([128, G, 2 * CN], BF16, tag="e")
            t1_0 = zwork.tile([128, 4, 2 * CN], BF16, tag="t1", bufs=1)
            score_stage(0, 0, 0, e0)
            score_stage(0, 0, 1, e0)
            pair_sum(1, e0, t1_0)
            k_half(0, 4)
            k_half(0, 6)
            q_part(1)
            score_stage(0, 0, 2, e0)
            score_stage(0, 0, 3, e0)
            pair_sum(3, e0, t1_0)
            for hc in range(2):
                v_chain(0, hc)
                v_chain(1, hc)
            allvec["on"] = False
            tree_finish(e0, t1_0)
            prev = (0, e0)

            # --- supertiles 1..7: per-stage interleave.
            # k-part halves: chunk c at supertile c (sts 1-3), one 2-group
            # half per stage. V chains: own m-tiles, (2st+i%2, hc=i//2).
            for st in range(1, ST):
                e_s = ework.tile([128, G, 2 * CN], BF16, tag="e")
                t1 = zwork.tile([128, 4, 2 * CN], BF16, tag="t1", bufs=1)
                ep = None
                t1p = None
                if ST - NPRE <= st < 7:
                    pool = epre1 if st < 4 else epre2
                    ep = pool.tile([128, G, 2 * CN], BF16, tag="epre")
                    t1p = zwork.tile([128, 4, 2 * CN], BF16, tag="t1p",
                                     bufs=1)
                for i in range(4):
                    half, sub = divmod(i, 2)
                    score_stage(0, st, i, e_s)
                    # stage 0-1 V evacs on scalar (free early in the
                    # supertile), 2-3 on vector (queued after this
                    # supertile's own tree, so nothing waits on them)
                    # st7 defers its mt15 chains to the pass
                    # transition: they become PE cover for the
                    # exp(3)->tree->mul latency the pass-0 drain pieces
                    # wait on, and st7's exps clear scalar ~2us earlier.
                    if st < 7 or i % 2 == 0:
                        v_chain(2 * st + (i % 2), i // 2, evac_vec=False)
                    if st <= 3:
                        k_half(st, 2 * i)
                    ctx_piece(prev[0], prev[1], ctx_acc, half, sub,
                              firsts=(prev[0] == 0 and sub == 0))
                    pair_sum(i, e_s, t1)
                    if ep is not None:
                        score_stage(1, st, i, ep)
                        pair_sum(i, ep, t1p)
                tree_finish(e_s, t1)
                if ep is not None:
                    tree_finish(ep, t1p)
                    epre[st] = ep
                prev = (st, e_s)
                if st == 3:
                    # fp8 projection operands are dead once the last K
                    # part is emitted; recycle their SBUF for the
                    # retained pass-1 E tiles of supertiles 4-7.
                    proj8.release()
                    epre2 = tc.alloc_tile_pool(name="epre2", bufs=4)

            # =========== pass 0 drain + pass 1, interleaved ===========
            # Fresh pass-1 chains: supertiles 0 and 1. Their exp latency
            # is covered by the pass-0 drain (st7 ctx + evac + out DMA)
            # and the precomputed supertiles' ready ctx matmuls.
            f0e = ework.tile([128, G, 2 * CN], BF16, tag="e")
            f0t1 = zwork.tile([128, 4, 2 * CN], BF16, tag="t1", bufs=1)

            # fresh chain 0 + the deferred pre-st7 chain carry the
            # pass-0 drain: the drain pieces wait on st7's softmax tree
            # (vector), so the PE needs ~2us of independent score work
            # ahead of them.
            p7e = epre2.tile([128, G, 2 * CN], BF16, tag="epre")
            p7t1 = zwork.tile([128, 4, 2 * CN], BF16, tag="t1p", bufs=1)
            v_chain(15, 0, evac_vec=False)
            v_chain(15, 1, evac_vec=False)
            score_stage(1, 0, 0, f0e)
            score_stage(1, 0, 1, f0e)
            pair_sum(1, f0e, f0t1)
            score_stage(1, 7, 0, p7e)
            score_stage(1, 7, 1, p7e)
            pair_sum(1, p7e, p7t1)
            ctx_piece(prev[0], prev[1], ctx_acc, 0, 0, firsts=False)
            ctx_piece(prev[0], prev[1], ctx_acc, 0, 1, firsts=False,
                      stop=True)
            evac_out(0, 0, ctx_acc)
            ctx_piece(prev[0], prev[1], ctx_acc, 1, 0, firsts=False)
            ctx_piece(prev[0], prev[1], ctx_acc, 1, 1, firsts=False,
                      stop=True)
            evac_out(0, 1, ctx_acc)

            ctx_acc1 = cxp.tile([128, G, CN], F32, tag="cx")
            f1e = ework.tile([128, G, 2 * CN], BF16, tag="e")
            f1t1 = zwork.tile([128, 4, 2 * CN], BF16, tag="t1", bufs=1)

            # pass-1 fill queue: pieces (st, e, half, sub), pre supertiles
            # first (ready), fresh 0 after its tree, fresh 1 in the tail.
            # order: ready pre supertiles, then fresh 0 (tree done
            # mid-drain), then pre-st7 (tree just after), f1 last --
            # by the time its pieces emit, its tree has finished.
            fills = [(s, epre[s], half, sub)
                     for s in range(2, 7)
                     for half in range(2) for sub in range(2)]
            fills += [(0, f0e, half, sub) for half in range(2)
                      for sub in range(2)]
            fills += [(7, p7e, half, sub) for half in range(2)
                      for sub in range(2)]
            fills += [(1, f1e, half, sub) for half in range(2)
                      for sub in range(2)]
            nfill = [0]
            first1 = {0: True, 1: True}
            last_of_gh = {}
            for idx, (s_, e_, half, sub) in enumerate(fills):
                last_of_gh[half] = idx

            def fill_n(k):
                for _ in range(k):
                    if nfill[0] >= len(fills):
                        return
                    idx = nfill[0]
                    s_, e_, half, sub = fills[idx]
                    ctx_piece(s_, e_, ctx_acc1, half, sub,
                              firsts=first1[half],
                              stop=(idx == last_of_gh[half]))
                    first1[half] = False
                    nfill[0] += 1

            score_stage(1, 0, 2, f0e)
            score_stage(1, 7, 2, p7e)
            fill_n(2)
            score_stage(1, 0, 3, f0e)
            pair_sum(3, f0e, f0t1)
            score_stage(1, 7, 3, p7e)
            pair_sum(3, p7e, p7t1)
            fill_n(2)
            tree_finish(f0e, f0t1)
            tree_finish(p7e, p7t1)
            # fresh chain 1
            for i in range(4):
                score_stage(1, 1, i, f1e)
                pair_sum(i, f1e, f1t1)
                fill_n(3)
            tree_finish(f1e, f1t1)
            fill_n(len(fills))
            evac_out(1, 0, ctx_acc1)
            evac_out(1, 1, ctx_acc1)
            epre2.release()

    nc.compile()
    return nc


def _prep_inputs(hidden_states, Wq, bq, Wk, bk, Wv, bv):
    bf = ml_dtypes.bfloat16
    f8 = ml_dtypes.float8_e4m3
    # wv rearranged to the on-chip [p, t, o] layout (d = t*128 + p)
    wv_b = np.ascontiguousarray(
        np.asarray(Wv, np.float32).reshape(8, 128, D).transpose(1, 0, 2)
    ).astype(bf)

    # Wq/Wk scaled fp8, rearranged [d, o] -> [p, t, o], then
    # software-interleaved for DoubleRowSwInterleave
    def prep_w8(W):
        w8 = (np.asarray(W, np.float32) * WS).astype(f8)
        return _sw_interleave(w8.reshape(8, 128, D).transpose(1, 0, 2))

    wq8i = prep_w8(Wq)
    wk8i = prep_w8(Wk)
    bqs = np.ascontiguousarray(
        (np.asarray(bq, np.float32) * SCALE).reshape(G, 128).T
    )
    bks = np.ascontiguousarray(np.asarray(bk, np.float32).reshape(G, 128).T)
    bvt = np.asarray(bv, np.float32).astype(bf).reshape(1, D)

    in_maps = []
    for core in range(8):
        b, j = divmod(core, 4)
        xt = np.asarray(hidden_states[b], np.float32).T  # (D, S)
        xt = np.roll(xt, -j * NQ, axis=1)                # queries first
        # chunk-major on-chip layout [mc, p, t, mcol] (d = t*128 + p)
        xtp = np.ascontiguousarray(
            xt.reshape(8, 128, 4, 512).transpose(2, 1, 0, 3)
        ).astype(bf)
        in_maps.append(
            {
                "xt": xtp,
                "xt8": (xtp.astype(np.float32) * XS).astype(f8),
                "wq8i": wq8i, "wk8i": wk8i, "wv": wv_b,
                "bqs": bqs, "bks": bks, "bvt": bvt,
            }
        )
    return in_maps


def kernel(hidden_states, Wq, bq, Wk, bk, Wv, bv, _trace=False, _tmpdir=None):
    if "nc" not in _CACHE:
        _CACHE["nc"] = _build()
    nc = _CACHE["nc"]
    in_maps = _prep_inputs(hidden_states, Wq, bq, Wk, bk, Wv, bv)
    res = run_bass_kernel_spmd(
        nc, in_maps, list(range(8)), trace=_trace,
        **({"tmpdir": _tmpdir} if _tmpdir else {}),
    )
    _CACHE["last_result"] = res
    out = np.empty((B, S, D), np.float32)
    for core in range(8):
        b, j = divmod(core, 4)
        # ctxT [np, gh, p, gl, n]: d = (gh*4+gl)*128 + p, row = np*CN + n
        ct = res.results[core]["ctxT"].astype(np.float32)
        blk = ct.transpose(0, 4, 1, 3, 2).reshape(NQ, D)
        out[b, j * NQ : (j + 1) * NQ, :] = blk
    return out


# revision 41
# speedup vs baseline: 1.0053x; 1.0053x over previous
"""GroupQueryAttention (softmax over the GROUP axis) on 8 trn2 NeuronCores.

Reference computation (B=2, S=2048, D=1024, G=8, h=128):
    q = hidden @ Wq + bq ; k = hidden @ Wk + bk ; v = hidden @ Wv + bv
    scores[b,n,m,g] = sum_h q[b,n,g,h] k[b,m,g,h] / sqrt(D)
    probs = softmax(scores, axis=g)            # couples groups per (n,m)
    ctx[b,n,g,h] = sum_m probs[b,n,m,g] v[b,m,g,h]

Sharding: 2 batches x 4 query-blocks of 512 = 8 cores. The softmax over
g is local per core. Each core recomputes its batch's full K,V to avoid
cross-core collectives (~60us ncfw latency floor on this fabric).

Precision: Q,K projections run fp8e4 DoubleRowSwInterleave (weights
pre-interleaved on host so the fast-weight-load path stays on) with
x*32 / W*1024 pre-scales; V projection, scores and ctx matmuls stay
bf16 (an fp8 V or fp8 probs error enters ctx linearly through
sum_m p*dv and blows the max-abs gate).

Schedule: all K/V production is interleaved per-STAGE with the softmax
pipeline: each supertile stage emits its 4 score matmuls, then fillers
(one V psum chain, a 2-group K part on sts 1-3, and the previous
supertile's ctx matmuls), so the PE never outruns-and-stalls-on the
exp chain (scalar) and the HAM clock gate stays warm; ~62 dummy
matmuls at boot bridge the PE-activity window across the DMA-bound
prologue. Pass-1 softmax for supertiles 2-6 is precomputed during
pass 0 into retained E tiles (SBUF recycled from the fp8 projection
operands once K production ends at st3). Supertile 7 defers its mt15
V chains and its pass-1 precompute into the pass transition, where
they cover the exp->tree->normalize latency that the pass-0 drain
pieces wait on; the two fresh pass-1 chains then interleave with the
precomputed supertiles' ready ctx matmuls as stage fillers.

Softmax runs on 2-m-tile supertiles (8 x 512 probs): exp + V/ctx PSUM
evacuations on Scalar, pair-sum tree half on GpSimd (SBUF-only engine,
otherwise idle), tree tail + normalize mul on Vector. Q/K projection
evacuations alternate Scalar/Vector (scalar_tensor_tensor does
scale+bias on Vector) -- all-Vector during supertile 0 where scalar is
the ramp bottleneck -- so no single engine's serial evac chain paces
the DRSW projection matmuls.

Output: ctxT (1024, 512) bf16 per core; host upcasts/transposes/concats.
"""

import os

os.environ.setdefault("JAX_COMPILATION_CACHE_DIR", "/tmp/jax_comp_cache")

import numpy as np
import ml_dtypes

import concourse.bass as bass
import concourse.mybir as mybir
import concourse.tile as tile
from concourse import bacc
from concourse.bass_utils import run_bass_kernel_spmd

BF16 = mybir.dt.bfloat16
F32 = mybir.dt.float32
FP8 = mybir.dt.float8e4
DRSW = mybir.MatmulPerfMode.DoubleRowSwInterleave

B, S, D, G = 2, 2048, 1024, 8
H = D // G          # 128, group head dim
NQ = S // 4         # 512 queries per core
MT = S // 128       # 16 key m-tiles
ST = MT // 2        # 8 supertiles (2 m-tiles each)
CN = 256            # n-chunk (queries per attention pass)
NP = NQ // CN       # 2 passes
NPRE = 6            # pass-1 supertiles precomputed during pass 0
SCALE = 1.0 / np.sqrt(np.float32(D))  # 1/32
XS = 32.0           # fp8 pre-scale on x
WS = 1024.0         # fp8 pre-scale on Wq/Wk
DESC = 1.0 / (XS * WS)  # 2^-15 descale for fp8 QK psums

_CACHE = {}


def _sw_interleave(w8):
    """Host layout for DoubleRowSwInterleave stationary operands.

    w8: [128, 8, 1024] fp8 (partition, k-subtile t, out-col o). Returns
    [128, 4, 8, 256]: per (k-subtile-pair cp, out-group g of 128 cols),
    columns stored reversed with the (A=even subtile, B=odd subtile)
    values interleaved per column: pos 2*(127-c) = A[c], 2*(127-c)+1 = B[c].
    """
    A = w8[:, 0::2, :].reshape(128, 4, 8, 128)   # [p, cp, g, c]
    Bm = w8[:, 1::2, :].reshape(128, 4, 8, 128)
    inter = np.stack([A[..., ::-1], Bm[..., ::-1]], axis=-1)  # [p,cp,g,128,2]
    return np.ascontiguousarray(inter.reshape(128, 4, 8, 256))


def _build():
    nc = bacc.Bacc()

    xt_d = nc.dram_tensor("xt", [4, 128, 8, 512], BF16, kind="ExternalInput")
    xt8_d = nc.dram_tensor("xt8", [4, 128, 8, 512], FP8, kind="ExternalInput")
    wq8_d = nc.dram_tensor("wq8i", [128, 4, G, 256], FP8, kind="ExternalInput")
    wk8_d = nc.dram_tensor("wk8i", [128, 4, G, 256], FP8, kind="ExternalInput")
    wv_d = nc.dram_tensor("wv", [128, 8, D], BF16, kind="ExternalInput")
    bqs_d = nc.dram_tensor("bqs", [128, G], F32, kind="ExternalInput")
    bks_d = nc.dram_tensor("bks", [128, G], F32, kind="ExternalInput")
    bvt_d = nc.dram_tensor("bvt", [1, D], BF16, kind="ExternalInput")
    out_d = nc.dram_tensor("ctxT", [NP, 2, 128, 4, CN], BF16,
                           kind="ExternalOutput")

    with tile.TileContext(nc) as tc:
        with (
            tc.tile_pool(name="big", bufs=1) as big,
            tc.tile_pool(name="small", bufs=1) as small,
            tc.tile_pool(name="ework", bufs=2) as ework,
            tc.tile_pool(name="epre1", bufs=2) as epre1,
            tc.tile_pool(name="zwork", bufs=2) as zwork,
            tc.tile_pool(name="sc", bufs=2, space="PSUM") as scp,
            tc.tile_pool(name="cx", bufs=1, space="PSUM") as cxp,
        ):
            proj8 = tc.alloc_tile_pool(name="proj8", bufs=1)
            xt_s = big.tile([128, 4, 8, 512], BF16)  # [p, mc, dt, mcol]
            xt8_s = proj8.tile([128, 4, 8, 512], FP8)
            wq8_s = proj8.tile([128, 4, G, 256], FP8)
            wk8_s = proj8.tile([128, 4, G, 256], FP8)
            wv_s = big.tile([128, 8, D], BF16)

            # ---- input DMA, 3 queues, ordered by first-use time.
            # sync: the fp8 projection operands (K/Q critical path);
            # scalar: K weights then the xt bf16 chunks V needs first;
            # gpsimd: biases + V weights, then the late xt bulk.
            nc.sync.dma_start(xt8_s[:, 0], xt8_d[0])
            nc.sync.dma_start(wq8_s[:, :, 0:4], wq8_d[:, :, 0:4])
            nc.sync.dma_start(wq8_s[:, :, 4:8], wq8_d[:, :, 4:8])
            nc.sync.dma_start(xt8_s[:, 1], xt8_d[1])
            nc.sync.dma_start(
                xt8_s[:, 2:4], xt8_d[2:4].rearrange("c p t m -> p c t m")
            )
            nc.scalar.dma_start(wk8_s[:, :, 0:2], wk8_d[:, :, 0:2])
            nc.scalar.dma_start(wk8_s[:, :, 2:4], wk8_d[:, :, 2:4])
            nc.scalar.dma_start(wk8_s[:, :, 4:8], wk8_d[:, :, 4:8])
            nc.scalar.dma_start(xt_s[:, 0], xt_d[0])
            nc.scalar.dma_start(xt_s[:, 2], xt_d[2])
            bqs_s = small.tile([128, G], F32)
            nc.gpsimd.dma_start(bqs_s[:], bqs_d[:])
            bks_s = small.tile([128, G], F32)
            nc.gpsimd.dma_start(bks_s[:], bks_d[:])
            bvt_s = small.tile([1, D], BF16)
            nc.gpsimd.dma_start(bvt_s[:], bvt_d[:])
            nc.gpsimd.dma_start(wv_s[:, :, 0:512], wv_d[:, :, 0:512])
            nc.gpsimd.dma_start(wv_s[:, :, 512:1024], wv_d[:, :, 512:1024])
            nc.gpsimd.dma_start(xt_s[:, 1], xt_d[1])
            nc.gpsimd.dma_start(xt_s[:, 3], xt_d[3])
            ones_s = small.tile([1, 128], BF16)
            nc.gpsimd.memset(ones_s[:], 1.0)

            # ---- HAM warmup: ~50 dummy matmuls on a memset tile while
            # the input DMA is in flight. The PE clock gate (HAM) needs
            # ~3.4us of sustained activity to release the 1.2->2.4 GHz
            # throttle; without this the first ~7us of real matmuls run
            # at half clock.
            warm_s = small.tile([128, 128], BF16)
            nc.vector.memset(warm_s[:], 0.0)
            warmp = scp.tile([128, 64], F32, tag="sc")
            for w in range(62):
                nc.tensor.matmul(
                    warmp[:], warm_s[:], warm_s[:, 0:64],
                    start=(w == 0), stop=(w == 61),
                )

            kt_s = big.tile([128, G, S], BF16)       # [h, g, m]
            v_s = big.tile([128, MT, D], BF16)       # [m, mt, g*128+h]
            qt_s = big.tile([128, G, NQ], BF16)      # [h, g, n]
            ctxt_s = big.tile([128, G, CN], BF16)    # [h, g, n] one pass

            ident = mybir.ActivationFunctionType.Identity
            expf = mybir.ActivationFunctionType.Exp
            mult = mybir.AluOpType.mult
            addop = mybir.AluOpType.add

            def bias_bcast(bt, g, n):
                # column g of a [128, G] bias tile, broadcast over n cols
                return bass.AP(
                    tensor=bt.tensor, offset=bt.offset + g,
                    ap=[bt.ap[0], [0, n]],
                )

            # ---- Q^T projection (queries are XT columns 0:NQ), fp8 DRSW -----
            # Projection PSUM evacuations alternate scalar/vector so
            # neither engine's serial evac chain paces the DRSW matmuls.
            # During supertile 0 the ctx-accumulator banks are still idle;
            # boot_psum hands out sub-slots of them for every other
            # projection chain, widening the psum rotation from 2 to ~6
            # so no chain waits on an evac in the DMA-limited ramp.
            boot = {"t": None, "i": 0}
            allvec = {"on": False}

            def proj_psum(n):
                if boot["t"] is not None:
                    i = boot["i"]
                    boot["i"] += 1
                    if i % 2 == 1:
                        return boot["t"][:, (i // 2) % 4, 0:n]
                return scp.tile([128, n], F32, tag="sc", name="projp")

            def q_part(gh):
                for g in range(gh * 4, gh * 4 + 4):
                    qp = proj_psum(NQ)
                    for cp in range(4):
                        nc.tensor.matmul(
                            qp[:],
                            wq8_s[:, cp, g, :],
                            xt8_s[:, 0, 2 * cp : 2 * cp + 2, :],
                            start=(cp == 0),
                            stop=(cp == 3),
                            perf_mode=DRSW,
                        )
                    if g % 2 == 0 and not allvec["on"]:
                        nc.scalar.activation(
                            qt_s[:, g, :], qp[:], ident,
                            bias=bqs_s[:, g : g + 1],
                            scale=float(SCALE * DESC),
                        )
                    else:
                        nc.vector.scalar_tensor_tensor(
                            qt_s[:, g, :], qp[:], float(SCALE * DESC),
                            bias_bcast(bqs_s, g, NQ), mult, addop,
                        )

            def k_half(mc, g0):
                # K^T columns mc*512..+512 for groups g0, g0+1
                for g in (g0, g0 + 1):
                    kp = proj_psum(512)
                    for cp in range(4):
                        nc.tensor.matmul(
                            kp[:],
                            wk8_s[:, cp, g, :],
                            xt8_s[:, mc, 2 * cp : 2 * cp + 2, :],
                            start=(cp == 0),
                            stop=(cp == 3),
                            perf_mode=DRSW,
                        )
                    if g % 2 == 0 and not allvec["on"]:
                        nc.scalar.activation(
                            kt_s[:, g, mc * 512 : (mc + 1) * 512], kp[:],
                            ident, bias=bks_s[:, g : g + 1],
                            scale=float(DESC),
                        )
                    else:
                        nc.vector.scalar_tensor_tensor(
                            kt_s[:, g, mc * 512 : (mc + 1) * 512], kp[:],
                            float(DESC), bias_bcast(bks_s, g, 512),
                            mult, addop,
                        )

            def v_chain(mt, hc, evac_vec=True):  # noqa: D401
                # V rows for one (m-tile, 512-col half); +bv via a rank-1
                # ones matmul into the f32 PSUM (adding after the bf16
                # round would double the V quantization noise, which the
                # sum_m p*dv amplification turns into a gate failure)
                vp = scp.tile([128, 512], F32, tag="sc")
                for dt in range(8):
                    nc.tensor.matmul(
                        vp[:],
                        xt_s[:, mt // 4, dt,
                             (mt % 4) * 128 : (mt % 4) * 128 + 128],
                        wv_s[:, dt, hc * 512 : (hc + 1) * 512],
                        start=(dt == 0),
                        stop=False,
                    )
                nc.tensor.matmul(
                    vp[:],
                    ones_s[:],
                    bvt_s[:, hc * 512 : (hc + 1) * 512],
                    start=False,
                    stop=True,
                )
                # evac on vector: scalar's exp chain is the critical path
                # in the steady state and an extra 0.7us there surfaces
                # as a PE psum-rotation stall. st7 uses scalar instead so
                # the vector queue reaches st7's softmax tree sooner (the
                # pass-0 drain waits on it).
                if evac_vec:
                    nc.vector.tensor_copy(
                        v_s[:, mt, hc * 512 : (hc + 1) * 512], vp[:]
                    )
                else:
                    nc.scalar.activation(
                        v_s[:, mt, hc * 512 : (hc + 1) * 512], vp[:], ident
                    )

            def score_stage(np_, st, i, e_s):
                # 4 score matmuls + exp for stage i = (half, sub) of a
                # supertile against n-chunk np_.
                half, sub = divmod(i, 2)
                n0 = np_ * CN
                mt = 2 * st + sub
                sp = scp.tile([128, 4, CN], F32, tag="sc")
                for gl in range(4):
                    g = half * 4 + gl
                    nc.tensor.matmul(
                        sp[:, gl, :],
                        kt_s[:, g, mt * 128 : (mt + 1) * 128],
                        qt_s[:, g, n0 : n0 + CN],
                        start=True,
                        stop=True,
                    )
                nc.scalar.activation(
                    e_s[:, half * 4 : half * 4 + 4, sub * CN : (sub + 1) * CN],
                    sp[:], expf,
                )

            def pair_sum(i, e_s, t1):
                # pair-sums: half 0 on gpsimd (slack before t2 needs it),
                # half 1 on vector (fast, feeds t2 immediately)
                if i == 1:
                    nc.gpsimd.tensor_add(
                        t1[:, 0:2, :], e_s[:, 0:2, :], e_s[:, 2:4, :]
                    )
                elif i == 3:
                    nc.vector.tensor_add(
                        t1[:, 2:4, :], e_s[:, 4:6, :], e_s[:, 6:8, :]
                    )

            def tree_finish(e_s, t1):
                t2 = zwork.tile([128, 2, 2 * CN], BF16, tag="t2", bufs=1)
                nc.vector.tensor_add(t2[:], t1[:, 0:2, :], t1[:, 2:4, :])
                z32 = zwork.tile([128, 2 * CN], F32, tag="z32", bufs=1)
                nc.vector.tensor_add(z32[:], t2[:, 0, :], t2[:, 1, :])
                nc.vector.reciprocal_approx_fast(out=z32[:], in_=z32[:])
                wb = zwork.tile([128, 2 * CN], BF16, tag="wb", bufs=1)
                nc.vector.tensor_copy(wb[:], z32[:])
                # normalize per sub-tile so ctx matmuls on sub 0 can start
                # while sub 1 is still being scaled
                for sub in range(2):
                    wb_b = bass.AP(
                        tensor=wb.tensor, offset=wb.offset + sub * CN,
                        ap=[wb.ap[0], [0, G], [1, CN]],
                    )
                    nc.vector.tensor_mul(
                        e_s[:, :, sub * CN : (sub + 1) * CN],
                        e_s[:, :, sub * CN : (sub + 1) * CN],
                        wb_b,
                    )

            # ctx^T accumulation: out[h, n] += V_g^T @ P_g^T
            # Two groups share each 2KB PSUM bank. start=True resets the
            # whole bank's has_written bits, so only the first group of
            # each bank pair may issue it; the second group's first write
            # lands on cleared bits and overwrites, later writes accumulate.
            def ctx_piece(st_, e_, ctx_acc, half, sub, firsts, stop=False):
                mt = 2 * st_ + sub
                for g in range(half * 4, half * 4 + 4):
                    nc.tensor.matmul(
                        ctx_acc[:, g, :],
                        v_s[:, mt, g * 128 : (g + 1) * 128],
                        e_[:, g, sub * CN : (sub + 1) * CN],
                        start=(firsts and g % 2 == 0),
                        stop=stop,
                        skip_group_check=True,
                    )

            def evac_out(np_, gh, ctx_acc):
                nc.scalar.activation(
                    ctxt_s[:, gh * 4 : gh * 4 + 4, :],
                    ctx_acc[:, gh * 4 : gh * 4 + 4, :], ident,
                )
                nc.sync.dma_start(
                    out_d[np_, gh], ctxt_s[:, gh * 4 : gh * 4 + 4, :]
                )

            # =========== pass 0 ===========
            epre = {}

            # --- supertile 0 (special: projections are the fillers; V
            # chains go last so a late wv DMA can't block the PE queue)
            ctx_acc = cxp.tile([128, G, CN], F32, tag="cx")
            # st0: ALL Q/K evacs on vector -- scalar is the ramp
            # bottleneck (exps + V evacs + DMA issue), vector is idle
            allvec["on"] = True
            k_half(0, 0)
            k_half(0, 2)
            q_part(0)
            e0 = ework.tile([128, G, 2 * CN], BF16, tag="e")
            t1_0 = zwork.tile([128, 4, 2 * CN], BF16, tag="t1", bufs=1)
            score_stage(0, 0, 0, e0)
            score_stage(0, 0, 1, e0)
            pair_sum(1, e0, t1_0)
            k_half(0, 4)
            k_half(0, 6)
            q_part(1)
            score_stage(0, 0, 2, e0)
            score_stage(0, 0, 3, e0)
            pair_sum(3, e0, t1_0)
            for hc in range(2):
                v_chain(0, hc)
                v_chain(1, hc)
            allvec["on"] = False
            tree_finish(e0, t1_0)
            prev = (0, e0)

            # --- supertiles 1..7: per-stage interleave.
            # k-part halves: chunk c at supertile c (sts 1-3), one 2-group
            # half per stage. V chains: own m-tiles, (2st+i%2, hc=i//2).
            for st in range(1, ST):
                e_s = ework.tile([128, G, 2 * CN], BF16, tag="e")
                t1 = zwork.tile([128, 4, 2 * CN], BF16, tag="t1", bufs=1)
                ep = None
                t1p = None
                if ST - NPRE <= st < 7:
                    pool = epre1 if st < 4 else epre2
                    ep = pool.tile([128, G, 2 * CN], BF16, tag="epre")
                    t1p = zwork.tile([128, 4, 2 * CN], BF16, tag="t1p",
                                     bufs=1)
                for i in range(4):
                    half, sub = divmod(i, 2)
                    score_stage(0, st, i, e_s)
                    # stage 0-1 V evacs on scalar (free early in the
                    # supertile), 2-3 on vector (queued after this
                    # supertile's own tree, so nothing waits on them)
                    # st7 defers its mt15 chains to the pass
                    # transition: they become PE cover for the
                    # exp(3)->tree->mul latency the pass-0 drain pieces
                    # wait on, and st7's exps clear scalar ~2us earlier.
                    if st < 7 or i % 2 == 0:
                        v_chain(2 * st + (i % 2), i // 2, evac_vec=False)
                    if st <= 3:
                        k_half(st, 2 * i)
                    ctx_piece(prev[0], prev[1], ctx_acc, half, sub,
                              firsts=(prev[0] == 0 and sub == 0))
                    pair_sum(i, e_s, t1)
                    if ep is not None:
                        score_stage(1, st, i, ep)
                        pair_sum(i, ep, t1p)
                tree_finish(e_s, t1)
                if ep is not None:
                    tree_finish(ep, t1p)
                    epre[st] = ep
                prev = (st, e_s)
                if st == 3:
                    # fp8 projection operands are dead once the last K
                    # part is emitted; recycle their SBUF for the
                    # retained pass-1 E tiles of supertiles 4-7.
                    proj8.release()
                    epre2 = tc.alloc_tile_pool(name="epre2", bufs=4)

            # =========== pass 0 drain + pass 1, interleaved ===========
            # Fresh pass-1 chains: supertiles 0 and 1. Their exp latency
            # is covered by the pass-0 drain (st7 ctx + evac + out DMA)
            # and the precomputed supertiles' ready ctx matmuls.
            f0e = ework.tile([128, G, 2 * CN], BF16, tag="e")
            f0t1 = zwork.tile([128, 4, 2 * CN], BF16, tag="t1", bufs=1)

            # fresh chain 0 + the deferred pre-st7 chain carry the
            # pass-0 drain: the drain pieces wait on st7's softmax tree
            # (vector), so the PE needs ~2us of independent score work
            # ahead of them.
            p7e = epre2.tile([128, G, 2 * CN], BF16, tag="epre")
            p7t1 = zwork.tile([128, 4, 2 * CN], BF16, tag="t1p", bufs=1)
            v_chain(15, 0, evac_vec=False)
            v_chain(15, 1, evac_vec=False)
            score_stage(1, 0, 0, f0e)
            score_stage(1, 0, 1, f0e)
            pair_sum(1, f0e, f0t1)
            score_stage(1, 7, 0, p7e)
            score_stage(1, 7, 1, p7e)
            pair_sum(1, p7e, p7t1)
            ctx_piece(prev[0], prev[1], ctx_acc, 0, 0, firsts=False)
            ctx_piece(prev[0], prev[1], ctx_acc, 0, 1, firsts=False,
                      stop=True)
            evac_out(0, 0, ctx_acc)
            ctx_piece(prev[0], prev[1], ctx_acc, 1, 0, firsts=False)
            ctx_piece(prev[0], prev[1], ctx_acc, 1, 1, firsts=False,
                      stop=True)
            evac_out(0, 1, ctx_acc)

            ctx_acc1 = cxp.tile([128, G, CN], F32, tag="cx")
            f1e = ework.tile([128, G, 2 * CN], BF16, tag="e")
            f1t1 = zwork.tile([128, 4, 2 * CN], BF16, tag="t1", bufs=1)

            # pass-1 fill queue: pieces (st, e, half, sub), pre supertiles
            # first (ready), fresh 0 after its tree, fresh 1 in the tail.
            # order: ready pre supertiles, then fresh 0 (tree done
            # mid-drain), then pre-st7 (tree just after), f1 last --
            # by the time its pieces emit, its tree has finished.
            fills = [(s, epre[s], half, sub)
                     for s in range(2, 7)
                     for half in range(2) for sub in range(2)]
            fills += [(0, f0e, half, sub) for half in range(2)
                      for sub in range(2)]
            fills += [(7, p7e, half, sub) for half in range(2)
                      for sub in range(2)]
            fills += [(1, f1e, half, sub) for half in range(2)
                      for sub in range(2)]
            nfill = [0]
            first1 = {0: True, 1: True}
            last_of_gh = {}
            for idx, (s_, e_, half, sub) in enumerate(fills):
                last_of_gh[half] = idx

            def fill_n(k):
                for _ in range(k):
                    if nfill[0] >= len(fills):
                        return
                    idx = nfill[0]
                    s_, e_, half, sub = fills[idx]
                    ctx_piece(s_, e_, ctx_acc1, half, sub,
                              firsts=first1[half],
                              stop=(idx == last_of_gh[half]))
                    first1[half] = False
                    nfill[0] += 1

            score_stage(1, 0, 2, f0e)
            score_stage(1, 7, 2, p7e)
            fill_n(2)
            score_stage(1, 0, 3, f0e)
            pair_sum(3, f0e, f0t1)
            score_stage(1, 7, 3, p7e)
            pair_sum(3, p7e, p7t1)
            fill_n(2)
            tree_finish(f0e, f0t1)
            tree_finish(p7e, p7t1)
            # fresh chain 1
            for i in range(4):
                score_stage(1, 1, i, f1e)
                pair_sum(i, f1e, f1t1)
                fill_n(3)
            tree_finish(f1e, f1t1)
            fill_n(len(fills))
            evac_out(1, 0, ctx_acc1)
            evac_out(1, 1, ctx_acc1)
            epre2.release()

    nc.compile()
    return nc


def _prep_inputs(hidden_states, Wq, bq, Wk, bk, Wv, bv):
    bf = ml_dtypes.bfloat16
    f8 = ml_dtypes.float8_e4m3
    # wv rearranged to the on-chip [p, t, o] layout (d = t*128 + p)
    wv_b = np.ascontiguousarray(
        np.asarray(Wv, np.float32).reshape(8, 128, D).transpose(1, 0, 2)
    ).astype(bf)

    # Wq/Wk scaled fp8, rearranged [d, o] -> [p, t, o], then
    # software-interleaved for DoubleRowSwInterleave
    def prep_w8(W):
        w8 = (np.asarray(W, np.float32) * WS).astype(f8)
        return _sw_interleave(w8.reshape(8, 128, D).transpose(1, 0, 2))

    wq8i = prep_w8(Wq)
    wk8i = prep_w8(Wk)
    bqs = np.ascontiguousarray(
        (np.asarray(bq, np.float32) * SCALE).reshape(G, 128).T
    )
    bks = np.ascontiguousarray(np.asarray(bk, np.float32).reshape(G, 128).T)
    bvt = np.asarray(bv, np.float32).astype(bf).reshape(1, D)

    in_maps = []
    for core in range(8):
        b, j = divmod(core, 4)
        xt = np.asarray(hidden_states[b], np.float32).T  # (D, S)
        xt = np.roll(xt, -j * NQ, axis=1)                # queries first
        # chunk-major on-chip layout [mc, p, t, mcol] (d = t*128 + p)
        xtp = np.ascontiguousarray(
            xt.reshape(8, 128, 4, 512).transpose(2, 1, 0, 3)
        ).astype(bf)
        in_maps.append(
            {
                "xt": xtp,
                "xt8": (xtp.astype(np.float32) * XS).astype(f8),
                "wq8i": wq8i, "wk8i": wk8i, "wv": wv_b,
                "bqs": bqs, "bks": bks, "bvt": bvt,
            }
        )
    return in_maps


def kernel(hidden_states, Wq, bq, Wk, bk, Wv, bv, _trace=False, _tmpdir=None):
    if "nc" not in _CACHE:
        _CACHE["nc"] = _build()
    nc = _CACHE["nc"]
    in_maps = _prep_inputs(hidden_states, Wq, bq, Wk, bk, Wv, bv)
    res = run_bass_kernel_spmd(
        nc, in_maps, list(range(8)), trace=_trace,
        **({"tmpdir": _tmpdir} if _tmpdir else {}),
    )
    _CACHE["last_result"] = res
    out = np.empty((B, S, D), np.float32)
    for core in range(8):
        b, j = divmod(core, 4)
        # ctxT [np, gh, p, gl, n]: d = (gh*4+gl)*128 + p, row = np*CN + n
        ct = res.results[core]["ctxT"].astype(np.float32)
        blk = ct.transpose(0, 4, 1, 3, 2).reshape(NQ, D)
        out[b, j * NQ : (j + 1) * NQ, :] = blk
    return out
